# revision 9
# baseline (speedup 1.0000x reference)
"""Trainium2 Bass kernel for nn_Net_SDE: 48-step neural SDE Monte-Carlo pricer.

Data-parallel over 131072 MC samples across 8 NeuronCores (16384/core).

v2 design (vs baseline): software-pipelined so the PE never waits on the
activation engines, activations spread across ACT/DVE/Pool, output layer
computed with swapped matmul operands (stationary = h3 chunk, moving = Wo
column) so the per-net scalar outputs land directly in state layout in PSUM
(no drain DMAs), fully unrolled 48-step loop.

Layouts (per core):
  - "column" C in [0, 16384) indexes a sample within the step's MLP batch.
  - state tiles S,V,S16,V16 are [128, 128] in T-layout: [d, m] = column
    C = m*128 + d. The swapped output matmul for chunk j (columns
    128j..128j+128) writes po[:, j] which is exactly this layout.
  - MLP activations: [feature, column] fp16, 1024-wide groups (16 groups).
  - layer-1 rhs inp [3, 16384] fp16 rows = (S-cS, V-cV, ones); bias folded
    into the ones row weight, so layer-1 act is a pure relu. inp rows are
    rebuilt each step half via XBAR transpose of S16/V16 + chunked flatten
    DMAs (partition-major linearization gives column order).
  - per-(maturity,strike) payoff partial sums per engine accumulate into
    three [128, 960] tiles (one per engine to avoid cross-engine WAW
    serialization); host sums the 8 cores x 3 tiles.
"""
import numpy as np
from contextlib import ExitStack

import orjson

import concourse.bass as bass
import concourse.tile as tile
from concourse import mybir

F16 = mybir.dt.float16
F32 = mybir.dt.float32
AF = mybir.ActivationFunctionType
OP = mybir.AluOpType

MC = 131072
N_STEPS = 48
N_CORES = 8
MCL = MC // N_CORES          # 16384 samples per core
N_GRP = MCL // 1024          # 16 groups of 1024 columns

STRIKES_CALL = np.array([100., 105., 110., 115., 120., 125., 130., 135., 140., 145.], np.float32)
STRIKES_PUT = np.array([55., 60., 65., 70., 75., 80., 85., 90., 95., 100.], np.float32)


# ---------------------------------------------------------------------------
# Workaround: this walrus build accepts only ONE sync-wait command per
# instruction. Split any instruction with more waits into preceding
# same-engine Drain (ctrl no-op) instructions, one wait each — same-engine
# FIFO order makes this semantically identical.
def _split_sync_waits(bir_json: bytes) -> bytes:
    bir = orjson.loads(bir_json)
    for fn in bir.get("functions", []):
        for bb in fn.get("blocks", []):
            out = []
            changed = False
            for ins in bb.get("instructions", []):
                si = ins.get("sync_info") or {}
                waits = si.get("on_wait") or []
                if len(waits) > 1:
                    changed = True
                    for ci, w in enumerate(waits[:-1]):
                        out.append({
                            "name": f"{ins['name']}_sw{ci}",
                            "opcode": "Drain",
                            "engine": ins.get("engine", "SP"),
                            "ins": [], "outs": [],
                            "debug": ins.get("debug"),
                            "sync_info": {"on_update": [], "on_wait": [w]},
                        })
                    si["on_wait"] = waits[-1:]
                    ins["sync_info"] = si
                out.append(ins)
            if changed:
                bb["instructions"] = out
    return orjson.dumps(bir)


def _install_sync_split():
    import concourse.bass_utils as bu
    import concourse.bass2jax as b2j
    if getattr(bu, "_sync_split_installed", False):
        return
    orig = bu.compile_bir_kernel

    def patched(bir_json, tmpdir, neff_name="file.neff"):
        return orig(_split_sync_waits(bir_json), tmpdir, neff_name=neff_name)

    bu.compile_bir_kernel = patched
    bu._sync_split_installed = True
    if getattr(b2j, "compile_bir_kernel", None) is orig:
        b2j.compile_bir_kernel = patched


# GPSIMD/Pool cannot access PSUM on TRN2, so PSUM->SBUF activation drains
# are split between ACT (~1070ns per [128,1024] tile) and DVE (~1237ns) by
# greedy deficit; Pool gets the SBUF-only work (state update, payoff).


def build_nc(idx_steps, c0, bo0, bo1h, bo2, bo3, cS, cV, n_steps=N_STEPS,
             repeat=1):
    """Build the single-core Bass program (SPMD: all cores run the same code)."""
    nc = bass.Bass()

    z_in = nc.declare_dram_parameter("z", [n_steps * 128, 128], F32, isOutput=False)
    z1_in = nc.declare_dram_parameter("z1", [n_steps * 128, 128], F32, isOutput=False)
    wiT3_in = nc.declare_dram_parameter("wiT3", [3, n_steps * 512], F16, isOutput=False)
    whT_in = nc.declare_dram_parameter("whT", [128, 1536], F16, isOutput=False)
    woT_in = nc.declare_dram_parameter("woT", [128, 4], F16, isOutput=False)
    bh_in = nc.declare_dram_parameter("bh", [128, 12], F32, isOutput=False)
    strk_in = nc.declare_dram_parameter("strk", [128, 40], F32, isOutput=False)
    accA_out = nc.declare_dram_parameter("accA", [128, 960], F32, isOutput=True)
    accD_out = nc.declare_dram_parameter("accD", [128, 960], F32, isOutput=True)
    accP_out = nc.declare_dram_parameter("accP", [128, 960], F32, isOutput=True)

    s_hist = nc.dram_tensor("s_hist", [n_steps * 128, 128], F32)

    with tile.TileContext(nc) as tc, ExitStack() as ctx:
        consts = ctx.enter_context(tc.tile_pool(name="consts", bufs=1))
        persist = ctx.enter_context(tc.tile_pool(name="persist", bufs=1))
        hpool = ctx.enter_context(tc.tile_pool(name="hpool", bufs=12))
        zpool = ctx.enter_context(tc.tile_pool(name="zpool", bufs=2))
        updpool = ctx.enter_context(tc.tile_pool(name="updpool", bufs=1))
        tailpool = ctx.enter_context(tc.tile_pool(name="tailpool", bufs=3))
        psmm = ctx.enter_context(tc.tile_pool(name="psmm", bufs=3, space="PSUM"))
        pspo = ctx.enter_context(tc.tile_pool(name="pspo", bufs=1, space="PSUM"))

        # constants
        wiT3 = consts.tile([3, n_steps * 512], F16)
        nc.sync.dma_start(out=wiT3, in_=wiT3_in[:, :])
        whT = consts.tile([128, 1536], F16)
        nc.sync.dma_start(out=whT, in_=whT_in[:, :])
        woT = consts.tile([128, 4], F16)
        nc.sync.dma_start(out=woT, in_=woT_in[:, :])
        bh = consts.tile([128, 12], F32)
        nc.sync.dma_start(out=bh, in_=bh_in[:, :])
        strk = consts.tile([128, 40], F32)
        nc.sync.dma_start(out=strk, in_=strk_in[:, :])
        zbias = consts.tile([128, 1], F32)
        nc.vector.memset(zbias[:, :], 0.0)

        # persistent state (T-layout)
        S = persist.tile([128, 128], F32)
        V = persist.tile([128, 128], F32)
        S16 = persist.tile([128, 128], F16)
        V16 = persist.tile([128, 128], F16)
        S16c = persist.tile([128, 128], F16)   # transposed copies for flatten
        V16c = persist.tile([128, 128], F16)
        inp = persist.tile([3, MCL], F16)
        accA = persist.tile([128, 960], F32)
        accD = persist.tile([128, 960], F32)
        accP = persist.tile([128, 960], F32)
        po = pspo.tile([128, 512], F32, tag="po")

        nc.vector.memset(S[:, :], cS)
        nc.vector.memset(V[:, :], cV)
        nc.vector.memset(S16[:, :], 0.0)
        nc.vector.memset(V16[:, :], 0.0)
        nc.vector.memset(S16c[:, :], 0.0)
        nc.vector.memset(V16c[:, :], 0.0)
        nc.vector.memset(inp[0:3, :], 1.0)     # ones row (bias input) ...
        nc.vector.memset(inp[0:2, :], 0.0)     # ... then centered state: S0-cS = 0
        nc.vector.memset(accA[:, :], 0.0)
        nc.vector.memset(accD[:, :], 0.0)
        nc.vector.memset(accP[:, :], 0.0)
        nc.vector.memset(po[:, :], 0.0)

        upd = []
        for i in range(3):
            upd_t = updpool.tile([128, 64], F32, tag=f"upd{i}", name=f"upd{i}")
            upd.append(upd_t)

        eng_t = {'A': 0.0, 'D': 0.0}

        def apply_act(h_new, pm, bias_ap):
            """bias+relu PSUM->SBUF on ACT or DVE, greedy load balance."""
            e = 'A' if eng_t['A'] + 1070 <= eng_t['D'] + 1237 else 'D'
            eng_t[e] += 1070 if e == 'A' else 1237
            if e == 'A':
                ap = zbias[:, 0:1] if bias_ap is None else bias_ap
                nc.scalar.activation(h_new, pm[:, :], AF.Relu, bias=ap, scale=1.0)
            else:
                if bias_ap is None:
                    nc.vector.tensor_scalar(out=h_new, in0=pm[:, :], scalar1=0.0,
                                            scalar2=None, op0=OP.max)
                else:
                    nc.vector.tensor_scalar(out=h_new, in0=pm[:, :], scalar1=bias_ap,
                                            scalar2=0.0, op0=OP.add, op1=OP.max)

        def do_update(half):
            """State update for columns m in [64*half, 64*half+64), T-layout."""
            cs = slice(64 * half, 64 * half + 64)
            po0 = po[:, 0 * 128 + 64 * half: 0 * 128 + 64 * half + 64]
            po1 = po[:, 1 * 128 + 64 * half: 1 * 128 + 64 * half + 64]
            po2 = po[:, 2 * 128 + 64 * half: 2 * 128 + 64 * half + 64]
            po3 = po[:, 3 * 128 + 64 * half: 3 * 128 + 64 * half + 64]
            zs = z_t[:, cs]
            z1s = z1_t[:, cs]
            # PSUM-reading and scalar_tensor_tensor ops must be DVE
            # (Pool can't access PSUM and only lowers plain tensor ops)
            nc.vector.scalar_tensor_tensor(out=upd[0], in0=po0, scalar=bo0,
                                           in1=zs, op0=OP.add, op1=OP.mult)
            nc.vector.scalar_tensor_tensor(out=upd[2], in0=po1, scalar=bo1h,
                                           in1=V[:, cs], op0=OP.add, op1=OP.add)
            # S_new = relu(c0*S + (diff+bo0)*dW)
            nc.vector.scalar_tensor_tensor(out=upd[1], in0=S[:, cs], scalar=c0,
                                           in1=upd[0], op0=OP.mult, op1=OP.add)
            nc.gpsimd.tensor_scalar(out=S[:, cs], in0=upd[1], scalar1=0.0,
                                    scalar2=None, op0=OP.max)
            nc.vector.scalar_tensor_tensor(out=upd[0], in0=po2, scalar=bo2,
                                           in1=zs, op0=OP.add, op1=OP.mult)
            nc.vector.scalar_tensor_tensor(out=upd[1], in0=po3, scalar=bo3,
                                           in1=z1s, op0=OP.add, op1=OP.mult)
            # V_new = V + (driftV*h+bo1h) + (diffV+bo2)*dW + (diffV1+bo3)*dW1
            nc.gpsimd.tensor_tensor(out=V[:, cs], in0=upd[2], in1=upd[0], op=OP.add)
            nc.gpsimd.tensor_tensor(out=V[:, cs], in0=V[:, cs], in1=upd[1], op=OP.add)
            # centered fp16 copies for the next step's layer-1 input
            nc.gpsimd.tensor_scalar(out=S16[:, cs], in0=S[:, cs], scalar1=cS,
                                    scalar2=None, op0=OP.subtract)
            nc.gpsimd.tensor_scalar(out=V16[:, cs], in0=V[:, cs], scalar1=cV,
                                    scalar2=None, op0=OP.subtract)
            # rebuild inp rows for these columns: transpose then flatten
            nc.sync.dma_start_transpose(out=S16c, in_=S16[:, :])
            nc.sync.dma_start_transpose(out=V16c, in_=V16[:, :])
            for q in range(4):
                pr = slice(64 * half + 16 * q, 64 * half + 16 * q + 16)
                fc = slice(8192 * half + 2048 * q, 8192 * half + 2048 * q + 2048)
                eng = nc.sync if q % 2 == 0 else nc.scalar
                eng.dma_start(out=inp[0:1, fc], in_=S16c[pr, :])
                eng.dma_start(out=inp[1:2, fc], in_=V16c[pr, :])

        # ---- main SDE loop (python-unrolled; repeat>1 is a timing-only mode) ----
        rep_ctx = (tc.For_i(0, repeat, 1) if repeat > 1 else None)
        if rep_ctx is not None:
            rep_ctx.__enter__()
        for t in range(n_steps):
            z_t = zpool.tile([128, 128], F32, tag="z")
            nc.sync.dma_start(out=z_t, in_=z_in[128 * t:128 * (t + 1), :])
            z1_t = zpool.tile([128, 128], F32, tag="z1")
            nc.sync.dma_start(out=z1_t, in_=z1_in[128 * t:128 * (t + 1), :])

            for half in range(2):
                for blk in range(4):
                    g0 = half * 8 + blk * 2
                    lanes = [(g, n) for g in (g0, g0 + 1) for n in range(4)]
                    h_cur = {}
                    for l in range(4):
                        for lane in lanes:
                            g, n = lane
                            pm = psmm.tile([128, 1024], F32, tag="pm")
                            if l == 0:
                                lhs = wiT3[:, (t * 4 + n) * 128:(t * 4 + n) * 128 + 128]
                                for hlf in range(2):
                                    nc.tensor.matmul(
                                        pm[:, hlf * 512:hlf * 512 + 512], lhsT=lhs,
                                        rhs=inp[:, g * 1024 + hlf * 512: g * 1024 + hlf * 512 + 512],
                                        start=True, stop=True)
                                bias_ap = None
                            else:
                                k = n * 3 + (l - 1)
                                lhs = whT[:, k * 128:(k + 1) * 128]
                                hp = h_cur[lane]
                                for hlf in range(2):
                                    sl = slice(hlf * 512, hlf * 512 + 512)
                                    nc.tensor.matmul(pm[:, sl], lhsT=lhs, rhs=hp[:, sl],
                                                     start=True, stop=True)
                                bias_ap = bh[:, k:k + 1]
                            h_new = hpool.tile([128, 1024], F16, tag="h")
                            apply_act(h_new, pm, bias_ap)
                            h_cur[lane] = h_new
                    # output layer: swapped matmul, lands in state layout
                    for lane in lanes:
                        g, n = lane
                        h3 = h_cur[lane]
                        for jj in range(8):
                            j = 8 * g + jj
                            nc.tensor.matmul(po[:, n * 128 + j:n * 128 + j + 1],
                                             lhsT=h3[:, 128 * jj:128 * jj + 128],
                                             rhs=woT[:, n:n + 1],
                                             start=True, stop=True)
                do_update(half)
            # save S trajectory (T-layout; payoff sums are order-invariant)
            nc.sync.dma_start(out=s_hist[128 * t:128 * (t + 1), :], in_=S[:, :])
        if rep_ctx is not None:
            rep_ctx.__exit__(None, None, None)

        # ---- payoff phase (indices baked at trace time) ----
        # acc column i*40+j: j 0-9 relu(S-Kc), 10-19 relu(Kp-S),
        #                    20-29 relu(S-Kp), 30-39 relu(Kc-S)
        junkA = tailpool.tile([128, 128], F32, tag="junkA")
        pay_t = {'A': 0.0, 'D': 0.0}
        PAY_COST = {'A': 511.0, 'D': 303.0}
        AX = mybir.AxisListType
        for i, step in enumerate(idx_steps):
            sh = tailpool.tile([128, 128], F32, tag="sh")
            nc.sync.dma_start(out=sh, in_=s_hist[128 * step:128 * (step + 1), :])
            nsh = tailpool.tile([128, 128], F32, tag="nsh")
            nc.gpsimd.tensor_scalar(out=nsh, in0=sh, scalar1=-1.0, scalar2=None,
                                    op0=OP.mult)
            # 21 reductions per maturity: relu(S-Kc) x10, relu(Kp-S) x10, sum(S).
            # relu(S-Kp) and relu(Kc-S) are derived on the host from these via
            # relu(x) - relu(-x) = x. ACT ops fuse relu+accumulate; DVE-assigned
            # ops use a Pool relu into tmp then a DVE free-axis reduce (DVE's
            # tensor_scalar accum_out does not accumulate).
            for j in range(21):
                e = min(pay_t, key=lambda k: pay_t[k] + PAY_COST[k])
                pay_t[e] += PAY_COST[e]
                acc = accA if e == 'A' else accD
                col = acc[:, i * 40 + j: i * 40 + j + 1]
                if e == 'A':
                    if j < 10:
                        nc.scalar.activation(junkA, sh, AF.Relu,
                                             bias=strk[:, j:j + 1], scale=1.0,
                                             accum_out=col)
                    elif j < 20:
                        nc.scalar.activation(junkA, sh, AF.Relu,
                                             bias=strk[:, j:j + 1], scale=-1.0,
                                             accum_out=col)
                    else:
                        nc.scalar.activation(junkA, sh, AF.Relu,
                                             bias=zbias[:, 0:1], scale=1.0,
                                             accum_out=col)
                else:
                    if j < 21 - 1:
                        tmp = tailpool.tile([128, 128], F32, tag="ptmp")
                        if j < 10:
                            K = float(STRIKES_CALL[j])
                            nc.gpsimd.tensor_scalar(out=tmp, in0=sh, scalar1=K,
                                                    scalar2=0.0, op0=OP.subtract,
                                                    op1=OP.max)
                        else:
                            K = float(STRIKES_PUT[j - 10])
                            nc.gpsimd.tensor_scalar(out=tmp, in0=nsh, scalar1=K,
                                                    scalar2=0.0, op0=OP.add,
                                                    op1=OP.max)
                        nc.vector.tensor_reduce(out=col, in_=tmp, axis=AX.X,
                                                op=OP.add)
                    else:
                        nc.vector.tensor_reduce(out=col, in_=sh, axis=AX.X,
                                                op=OP.add)
        nc.sync.dma_start(out=accA_out[:, :], in_=accA)
        nc.sync.dma_start(out=accD_out[:, :], in_=accD)
        nc.sync.dma_start(out=accP_out[:, :], in_=accP)

    return nc


def _prep_inputs(S0, V0, rate, z, z1, indices, timegrid, Wi, bi, Wh, bh, Wo, bo,
                 n_steps=N_STEPS):
    """Host-side preprocessing. Returns (nc build args, per-core input maps, disc)."""
    S0v = float(np.asarray(S0).reshape(-1)[0])
    V0v = float(np.asarray(V0).reshape(-1)[0])
    r = float(np.asarray(rate).reshape(-1)[0])
    tg = np.asarray(timegrid, np.float64)
    h = float(tg[1] - tg[0])
    sqh = float(np.sqrt(h))
    c0 = 1.0 + r * h

    Wi = np.asarray(Wi, np.float32)
    bi = np.asarray(bi, np.float32)
    Wh = np.asarray(Wh, np.float32)
    bhv = np.asarray(bh, np.float32)
    Wo = np.asarray(Wo, np.float32).copy()
    bo = np.asarray(bo, np.float32).copy()
    # driftV net (index 1) is only ever used multiplied by h -> fold h into it
    Wo[1] *= h
    bo0, bo1h, bo2, bo3 = float(bo[0, 0]), float(bo[1, 0]) * h, float(bo[2, 0]), float(bo[3, 0])

    cS, cV = S0v, V0v    # centering constants for fp16 inputs
    # first-layer bias with t-term and centering folded in: [4, T, 128]
    t_vals = tg[:n_steps].astype(np.float32)
    b1 = (bi[:, None, :] + t_vals[None, :, None] * Wi[:, 0][:, None, :]
          + cS * Wi[:, 1][:, None, :] + cV * Wi[:, 2][:, None, :])

    # layer-1 stationary per (t, n): rows (Wi_S, Wi_V, b1(t)): [3, T*4*128]
    wiT3 = np.empty((3, n_steps, 4, 128), np.float32)
    wiT3[0] = np.broadcast_to(Wi[:, 1, :][None, :, :], (n_steps, 4, 128))
    wiT3[1] = np.broadcast_to(Wi[:, 2, :][None, :, :], (n_steps, 4, 128))
    wiT3[2] = b1.transpose(1, 0, 2)
    wiT3_dev = np.ascontiguousarray(wiT3.reshape(3, n_steps * 512), np.float16)

    whT_dev = np.ascontiguousarray(
        Wh.transpose(2, 0, 1, 3).reshape(128, 12 * 128), np.float16)
    woT_dev = np.ascontiguousarray(Wo[:, :, 0].T, np.float16)
    bh_dev = np.ascontiguousarray(bhv.transpose(2, 0, 1).reshape(128, 12), np.float32)

    strk_dev = np.ascontiguousarray(
        np.tile(np.concatenate([-STRIKES_CALL, STRIKES_PUT,
                                -STRIKES_PUT, STRIKES_CALL])[None, :], (128, 1)),
        np.float32)

    idx = np.asarray(indices).astype(np.int64).reshape(-1)
    idx_steps = [int((v - 1) % n_steps) for v in idx]
    disc = np.exp(-r * 2.0 * idx.astype(np.float64) / n_steps).astype(np.float64)

    z = np.asarray(z, np.float32)
    z1 = np.asarray(z1, np.float32)
    in_maps = []
    for kk in range(N_CORES):
        sl = slice(kk * MCL, (kk + 1) * MCL)
        # T-layout per step: dev[t*128+d, m] = sqh * z[sample m*128+d, t]
        zc = (z[sl, :n_steps] * sqh).T.reshape(n_steps, 128, 128)
        z1c = (z1[sl, :n_steps] * sqh).T.reshape(n_steps, 128, 128)
        in_maps.append({
            "z": np.ascontiguousarray(zc.transpose(0, 2, 1).reshape(n_steps * 128, 128), np.float32),
            "z1": np.ascontiguousarray(z1c.transpose(0, 2, 1).reshape(n_steps * 128, 128), np.float32),
            "wiT3": wiT3_dev, "whT": whT_dev, "woT": woT_dev, "bh": bh_dev,
            "strk": strk_dev,
        })
    build_args = dict(idx_steps=idx_steps, c0=c0, bo0=bo0, bo1h=bo1h,
                      bo2=bo2, bo3=bo3, cS=cS, cV=cV, n_steps=n_steps)
    return build_args, in_maps, disc


def _combine(results, disc):
    """Sum per-core per-engine [128, 960] partials into the [96, 10] output.

    Device provides relu(S-Kc) (cols 0-9), relu(Kp-S) (cols 10-19) and
    sum(S) (col 20) per maturity; the other two payoff families follow from
    relu(x) - relu(-x) = x summed over samples.
    """
    total = np.zeros((128, 960), np.float64)
    for res in results:
        for k in ("accA", "accD", "accP"):
            total += np.asarray(res[k], np.float64)
    cols = total.sum(axis=0).reshape(24, 40)
    calls_c = cols[:, 0:10]
    puts_p = cols[:, 10:20]
    sumS = cols[:, 20]
    Kc = STRIKES_CALL.astype(np.float64)
    Kp = STRIKES_PUT.astype(np.float64)
    calls_p = puts_p + sumS[:, None] - MC * Kp[None, :]
    puts_c = calls_c - sumS[:, None] + MC * Kc[None, :]
    out = np.concatenate([calls_c, puts_p, calls_p, puts_c], axis=0) / MC
    out = out * np.concatenate([disc] * 4)[:, None]
    return out.astype(np.float32)


def kernel(**inputs) -> np.ndarray:
    from concourse.bass_utils import run_bass_kernel_spmd
    _install_sync_split()
    build_args, in_maps, disc = _prep_inputs(**inputs)
    nc = build_nc(**build_args)
    res = run_bass_kernel_spmd(nc, in_maps, list(range(N_CORES)))
    return _combine(res.results, disc)


# revision 12
# speedup vs baseline: 4.4158x; 4.4158x over previous
"""Trainium2 Bass kernel for nn_Net_SDE: 48-step neural SDE Monte-Carlo pricer.

Data-parallel over 131072 MC samples across 8 NeuronCores (16384/core).

v2 design (vs baseline): software-pipelined so the PE never waits on the
activation engines, activations spread across ACT/DVE/Pool, output layer
computed with swapped matmul operands (stationary = h3 chunk, moving = Wo
column) so the per-net scalar outputs land directly in state layout in PSUM
(no drain DMAs), fully unrolled 48-step loop.

Layouts (per core):
  - state tiles S,V,S16,V16 are [128, 128], sample = p*128 + f.
  - MLP activations: [feature, sample] fp16, 1024-wide groups (16 groups).
  - layer-1 rhs inp [3, 16384] fp16 rows = (S-cS, V-cV, ones); the
    time-dependent bias is folded into the ones-row weights, so the
    layer-1 act is a pure relu. inp rows are rebuilt per half-step by
    flatten DMAs (partition-major linearization gives sample order).
  - output layer: M=1 matmuls (tile_position column quadrants) into a po
    psum tile, drained via one DVE copy per group into orow, then 4 DMAs
    per half scatter rows back to the [128, 512] state-layout outs_sq.
  - per-(maturity,strike) payoff partial sums per engine accumulate into
    three [128, 960] tiles (one per engine to avoid cross-engine WAW
    serialization); host sums the 8 cores x 3 tiles.
"""
import numpy as np
from contextlib import ExitStack

import orjson

import concourse.bass as bass
import concourse.tile as tile
from concourse import mybir

F16 = mybir.dt.float16
F32 = mybir.dt.float32
AF = mybir.ActivationFunctionType
OP = mybir.AluOpType

MC = 131072
N_STEPS = 48
N_CORES = 8
MCL = MC // N_CORES          # 16384 samples per core
N_GRP = MCL // 1024          # 16 groups of 1024 columns

STRIKES_CALL = np.array([100., 105., 110., 115., 120., 125., 130., 135., 140., 145.], np.float32)
STRIKES_PUT = np.array([55., 60., 65., 70., 75., 80., 85., 90., 95., 100.], np.float32)


# ---------------------------------------------------------------------------
# Workaround: this walrus build accepts only ONE sync-wait command per
# instruction. Split any instruction with more waits into preceding
# same-engine Drain (ctrl no-op) instructions, one wait each — same-engine
# FIFO order makes this semantically identical.
def _split_sync_waits(bir_json: bytes) -> bytes:
    bir = orjson.loads(bir_json)
    for fn in bir.get("functions", []):
        for bb in fn.get("blocks", []):
            # Drop Ldweights whose operands exactly match the previous
            # Ldweights on the same engine (weights persist in the PE array
            # until the next load). Only sync-free ones are safe to remove.
            pruned = []
            last_ld = {}
            for ins in bb.get("instructions", []):
                op = ins.get("opcode")
                eng = ins.get("engine")
                if op == "Ldweights":
                    si = ins.get("sync_info") or {}
                    key = orjson.dumps(
                        [ins.get("ins"), ins.get("tile_position"),
                         ins.get("tile_size"), ins.get("perf_mode"),
                         ins.get("is_transpose")])
                    if (not si.get("on_wait") and not si.get("on_update")
                            and last_ld.get(eng) == key):
                        continue
                    last_ld[eng] = key
                elif op != "Matmult" and eng in last_ld:
                    # any other PE instruction invalidates tracking conservatively
                    if eng == "PE":
                        last_ld.pop(eng, None)
                pruned.append(ins)
            bb["instructions"] = pruned
            out = []
            changed = False
            for ins in bb.get("instructions", []):
                si = ins.get("sync_info") or {}
                waits = si.get("on_wait") or []
                if len(waits) > 1:
                    changed = True
                    for ci, w in enumerate(waits[:-1]):
                        out.append({
                            "name": f"{ins['name']}_sw{ci}",
                            "opcode": "Drain",
                            "engine": ins.get("engine", "SP"),
                            "ins": [], "outs": [],
                            "debug": ins.get("debug"),
                            "sync_info": {"on_update": [], "on_wait": [w]},
                        })
                    si["on_wait"] = waits[-1:]
                    ins["sync_info"] = si
                out.append(ins)
            if changed:
                bb["instructions"] = out
    return orjson.dumps(bir)


def _install_sync_split():
    import concourse.bass_utils as bu
    import concourse.bass2jax as b2j
    if getattr(bu, "_sync_split_installed", False):
        return
    orig = bu.compile_bir_kernel

    def patched(bir_json, tmpdir, neff_name="file.neff"):
        return orig(_split_sync_waits(bir_json), tmpdir, neff_name=neff_name)

    bu.compile_bir_kernel = patched
    bu._sync_split_installed = True
    if getattr(b2j, "compile_bir_kernel", None) is orig:
        b2j.compile_bir_kernel = patched


# GPSIMD/Pool cannot access PSUM on TRN2, so PSUM->SBUF activation drains
# are split between ACT (~1070ns per [128,1024] tile) and DVE (~1237ns) by
# greedy deficit; Pool gets the SBUF-only work (state update, payoff).


def build_nc(idx_steps, c0, bo0, bo1h, bo2, bo3, cS, cV, n_steps=N_STEPS,
             repeat=1):
    """Build the single-core Bass program (SPMD: all cores run the same code)."""
    nc = bass.Bass()

    z_in = nc.declare_dram_parameter("z", [n_steps * 128, 128], F32, isOutput=False)
    z1_in = nc.declare_dram_parameter("z1", [n_steps * 128, 128], F32, isOutput=False)
    wiT3_in = nc.declare_dram_parameter("wiT3", [3, n_steps * 512], F16, isOutput=False)
    whT_in = nc.declare_dram_parameter("whT", [128, 1536], F16, isOutput=False)
    woT_in = nc.declare_dram_parameter("woT", [128, 4], F16, isOutput=False)
    bh_in = nc.declare_dram_parameter("bh", [128, 12], F32, isOutput=False)
    strk_in = nc.declare_dram_parameter("strk", [128, 40], F32, isOutput=False)
    accA_out = nc.declare_dram_parameter("accA", [128, 960], F32, isOutput=True)
    accD_out = nc.declare_dram_parameter("accD", [128, 960], F32, isOutput=True)
    accP_out = nc.declare_dram_parameter("accP", [128, 960], F32, isOutput=True)

    s_hist = nc.dram_tensor("s_hist", [n_steps * 128, 128], F32)

    with tile.TileContext(nc) as tc, ExitStack() as ctx:
        consts = ctx.enter_context(tc.tile_pool(name="consts", bufs=1))
        persist = ctx.enter_context(tc.tile_pool(name="persist", bufs=1))
        hpool = ctx.enter_context(tc.tile_pool(name="hpool", bufs=12))
        zpool = ctx.enter_context(tc.tile_pool(name="zpool", bufs=2))
        updpool = ctx.enter_context(tc.tile_pool(name="updpool", bufs=1))
        tailpool = ctx.enter_context(tc.tile_pool(name="tailpool", bufs=3))
        psmm = ctx.enter_context(tc.tile_pool(name="psmm", bufs=3, space="PSUM"))
        pspo = ctx.enter_context(tc.tile_pool(name="pspo", bufs=1, space="PSUM"))

        # constants
        wiT3 = consts.tile([3, n_steps * 512], F16)
        nc.sync.dma_start(out=wiT3, in_=wiT3_in[:, :])
        whT = consts.tile([128, 1536], F16)
        nc.sync.dma_start(out=whT, in_=whT_in[:, :])
        woT = consts.tile([128, 4], F16)
        nc.sync.dma_start(out=woT, in_=woT_in[:, :])
        bh = consts.tile([128, 12], F32)
        nc.sync.dma_start(out=bh, in_=bh_in[:, :])
        strk = consts.tile([128, 40], F32)
        nc.sync.dma_start(out=strk, in_=strk_in[:, :])
        zbias = consts.tile([128, 1], F32)
        nc.vector.memset(zbias[:, :], 0.0)

        # persistent state (T-layout)
        S = persist.tile([128, 128], F32)
        V = persist.tile([128, 128], F32)
        S16 = persist.tile([128, 128], F16)
        V16 = persist.tile([128, 128], F16)
        inp = persist.tile([3, MCL], F16)
        orow = persist.tile([128, 8192], F32)   # out-layer rows, one half
        outs_sq = persist.tile([128, 512], F32)  # o_n in state layout
        accA = persist.tile([128, 960], F32)
        accD = persist.tile([128, 960], F32)
        accP = persist.tile([128, 960], F32)

        nc.vector.memset(S[:, :], cS)
        nc.vector.memset(V[:, :], cV)
        nc.vector.memset(S16[:, :], 0.0)
        nc.vector.memset(V16[:, :], 0.0)
        nc.vector.memset(inp[0:3, :], 1.0)     # ones row (bias input) ...
        nc.vector.memset(inp[0:2, :], 0.0)     # ... then centered state: S0-cS = 0
        nc.vector.memset(accA[:, :], 0.0)
        nc.vector.memset(accD[:, :], 0.0)
        nc.vector.memset(accP[:, :], 0.0)

        upd = []
        for i in range(3):
            upd_t = updpool.tile([128, 128], F32, tag=f"upd{i}", name=f"upd{i}")
            upd.append(upd_t)

        eng_t = {'A': 0.0, 'D': 0.0}

        def apply_act(h_new, pm, bias_ap):
            """bias+relu PSUM->SBUF on ACT or DVE, greedy load balance."""
            e = 'A' if eng_t['A'] + 1179 <= eng_t['D'] + 1348 else 'D'
            eng_t[e] += 1179 if e == 'A' else 1348
            if e == 'A':
                ap = zbias[:, 0:1] if bias_ap is None else bias_ap
                nc.scalar.activation(h_new, pm[:, :], AF.Relu, bias=ap, scale=1.0)
            else:
                if bias_ap is None:
                    nc.vector.tensor_scalar(out=h_new, in0=pm[:, :], scalar1=0.0,
                                            scalar2=None, op0=OP.max)
                else:
                    nc.vector.tensor_scalar(out=h_new, in0=pm[:, :], scalar1=bias_ap,
                                            scalar2=0.0, op0=OP.add, op1=OP.max)

        def do_update(half):
            """State update for partitions p in [64*half, 64*half+64)."""
            ps_ = slice(64 * half, 64 * half + 64)
            o0 = outs_sq[ps_, 0:128]
            o1 = outs_sq[ps_, 128:256]
            o2 = outs_sq[ps_, 256:384]
            o3 = outs_sq[ps_, 384:512]
            zs = z_t[ps_, :]
            z1s = z1_t[ps_, :]
            u0, u1, u2 = upd[0][ps_, :], upd[1][ps_, :], upd[2][ps_, :]
            nc.vector.scalar_tensor_tensor(out=u0, in0=o0, scalar=bo0,
                                           in1=zs, op0=OP.add, op1=OP.mult)
            nc.vector.scalar_tensor_tensor(out=u2, in0=o1, scalar=bo1h,
                                           in1=V[ps_, :], op0=OP.add, op1=OP.add)
            # S_new = relu(c0*S + (diff+bo0)*dW)
            nc.vector.scalar_tensor_tensor(out=u1, in0=S[ps_, :], scalar=c0,
                                           in1=u0, op0=OP.mult, op1=OP.add)
            nc.gpsimd.tensor_scalar(out=S[ps_, :], in0=u1, scalar1=0.0,
                                    scalar2=None, op0=OP.max)
            nc.vector.scalar_tensor_tensor(out=u0, in0=o2, scalar=bo2,
                                           in1=zs, op0=OP.add, op1=OP.mult)
            nc.vector.scalar_tensor_tensor(out=u1, in0=o3, scalar=bo3,
                                           in1=z1s, op0=OP.add, op1=OP.mult)
            # V_new = V + (driftV*h+bo1h) + (diffV+bo2)*dW + (diffV1+bo3)*dW1
            nc.gpsimd.tensor_tensor(out=V[ps_, :], in0=u2, in1=u0, op=OP.add)
            nc.gpsimd.tensor_tensor(out=V[ps_, :], in0=V[ps_, :], in1=u1, op=OP.add)
            # centered fp16 copies for the next step's layer-1 input
            nc.gpsimd.tensor_scalar(out=S16[ps_, :], in0=S[ps_, :], scalar1=cS,
                                    scalar2=None, op0=OP.subtract)
            nc.gpsimd.tensor_scalar(out=V16[ps_, :], in0=V[ps_, :], scalar1=cV,
                                    scalar2=None, op0=OP.subtract)
            # flatten: partition-major linearization = sample order
            fc = slice(8192 * half, 8192 * half + 8192)
            nc.sync.dma_start(out=inp[0:1, fc], in_=S16[ps_, :])
            nc.scalar.dma_start(out=inp[1:2, fc], in_=V16[ps_, :])

        # ---- main SDE loop (python-unrolled; repeat>1 is a timing-only mode) ----
        rep_ctx = (tc.For_i(0, repeat, 1) if repeat > 1 else None)
        if rep_ctx is not None:
            rep_ctx.__enter__()
        for t in range(n_steps):
            z_t = zpool.tile([128, 128], F32, tag="z")
            nc.sync.dma_start(out=z_t, in_=z_in[128 * t:128 * (t + 1), :])
            z1_t = zpool.tile([128, 128], F32, tag="z1")
            nc.sync.dma_start(out=z1_t, in_=z1_in[128 * t:128 * (t + 1), :])

            for half in range(2):
                for blk in range(4):
                    g0 = half * 8 + blk * 2
                    lanes = [(n, g) for n in range(4) for g in (g0, g0 + 1)]
                    h_cur = {}
                    for l in range(4):
                        for lane in lanes:
                            n, g = lane
                            pm = psmm.tile([128, 1024], F32, tag="pm")
                            if l == 0:
                                lhs = wiT3[:, (t * 4 + n) * 128:(t * 4 + n) * 128 + 128]
                                rhs_t, rbase = inp, g * 1024
                                bias_ap = None
                            else:
                                k = n * 3 + (l - 1)
                                lhs = whT[:, k * 128:(k + 1) * 128]
                                rhs_t, rbase = h_cur[lane], 0
                                bias_ap = bh[:, k:k + 1]
                            for hlf in range(2):
                                nc.tensor.matmul(
                                    pm[:, hlf * 512:hlf * 512 + 512], lhsT=lhs,
                                    rhs=rhs_t[:, rbase + hlf * 512: rbase + hlf * 512 + 512],
                                    start=True, stop=True)
                            h_new = hpool.tile([128, 1024], F16, tag="h")
                            apply_act(h_new, pm, bias_ap)
                            h_cur[lane] = h_new
                    # output layer: M=1 matmuls into po rows {0,32,64,96}
                    for g in (g0, g0 + 1):
                        po = psmm.tile([128, 1024], F32, tag="pm", name="po")
                        for n in range(4):
                            h3 = h_cur[(n, g)]
                            for hlf in range(2):
                                sl = slice(hlf * 512, hlf * 512 + 512)
                                nc.tensor.matmul(po[32 * n:32 * n + 1, sl],
                                                 lhsT=woT[:, n:n + 1], rhs=h3[:, sl],
                                                 start=True, stop=True,
                                                 tile_position=(0, 32 * n))
                        gl = g - 8 * half
                        nc.vector.tensor_copy(
                            orow[:, gl * 1024:(gl + 1) * 1024], po[:, :])
                        eng_t['D'] += 1348
                # scatter the 4 nets' rows into state layout, then update
                for n in range(4):
                    nc.sync.dma_start(
                        out=outs_sq[64 * half:64 * half + 64, 128 * n:128 * n + 128],
                        in_=orow[32 * n:32 * n + 1, :])
                do_update(half)
            # save S trajectory (payoff sums are order-invariant)
            nc.sync.dma_start(out=s_hist[128 * t:128 * (t + 1), :], in_=S[:, :])
        if rep_ctx is not None:
            rep_ctx.__exit__(None, None, None)

        # ---- payoff phase (indices baked at trace time) ----
        # acc column i*40+j: j 0-9 relu(S-Kc), 10-19 relu(Kp-S),
        #                    20-29 relu(S-Kp), 30-39 relu(Kc-S)
        junkA = tailpool.tile([128, 128], F32, tag="junkA")
        pay_t = {'A': 0.0, 'D': 0.0}
        PAY_COST = {'A': 511.0, 'D': 303.0}
        AX = mybir.AxisListType
        for i, step in enumerate(idx_steps):
            sh = tailpool.tile([128, 128], F32, tag="sh")
            nc.sync.dma_start(out=sh, in_=s_hist[128 * step:128 * (step + 1), :])
            nsh = tailpool.tile([128, 128], F32, tag="nsh")
            nc.gpsimd.tensor_scalar(out=nsh, in0=sh, scalar1=-1.0, scalar2=None,
                                    op0=OP.mult)
            # 21 reductions per maturity: relu(S-Kc) x10, relu(Kp-S) x10, sum(S).
            # relu(S-Kp) and relu(Kc-S) are derived on the host from these via
            # relu(x) - relu(-x) = x. ACT ops fuse relu+accumulate; DVE-assigned
            # ops use a Pool relu into tmp then a DVE free-axis reduce (DVE's
            # tensor_scalar accum_out does not accumulate).
            for j in range(21):
                e = min(pay_t, key=lambda k: pay_t[k] + PAY_COST[k])
                pay_t[e] += PAY_COST[e]
                acc = accA if e == 'A' else accD
                col = acc[:, i * 40 + j: i * 40 + j + 1]
                if e == 'A':
                    if j < 10:
                        nc.scalar.activation(junkA, sh, AF.Relu,
                                             bias=strk[:, j:j + 1], scale=1.0,
                                             accum_out=col)
                    elif j < 20:
                        nc.scalar.activation(junkA, sh, AF.Relu,
                                             bias=strk[:, j:j + 1], scale=-1.0,
                                             accum_out=col)
                    else:
                        nc.scalar.activation(junkA, sh, AF.Relu,
                                             bias=zbias[:, 0:1], scale=1.0,
                                             accum_out=col)
                else:
                    if j < 21 - 1:
                        tmp = tailpool.tile([128, 128], F32, tag="ptmp")
                        if j < 10:
                            K = float(STRIKES_CALL[j])
                            nc.gpsimd.tensor_scalar(out=tmp, in0=sh, scalar1=K,
                                                    scalar2=0.0, op0=OP.subtract,
                                                    op1=OP.max)
                        else:
                            K = float(STRIKES_PUT[j - 10])
                            nc.gpsimd.tensor_scalar(out=tmp, in0=nsh, scalar1=K,
                                                    scalar2=0.0, op0=OP.add,
                                                    op1=OP.max)
                        nc.vector.tensor_reduce(out=col, in_=tmp, axis=AX.X,
                                                op=OP.add)
                    else:
                        nc.vector.tensor_reduce(out=col, in_=sh, axis=AX.X,
                                                op=OP.add)
        nc.sync.dma_start(out=accA_out[:, :], in_=accA)
        nc.sync.dma_start(out=accD_out[:, :], in_=accD)
        nc.sync.dma_start(out=accP_out[:, :], in_=accP)

    return nc


def _prep_inputs(S0, V0, rate, z, z1, indices, timegrid, Wi, bi, Wh, bh, Wo, bo,
                 n_steps=N_STEPS):
    """Host-side preprocessing. Returns (nc build args, per-core input maps, disc)."""
    S0v = float(np.asarray(S0).reshape(-1)[0])
    V0v = float(np.asarray(V0).reshape(-1)[0])
    r = float(np.asarray(rate).reshape(-1)[0])
    tg = np.asarray(timegrid, np.float64)
    h = float(tg[1] - tg[0])
    sqh = float(np.sqrt(h))
    c0 = 1.0 + r * h

    Wi = np.asarray(Wi, np.float32)
    bi = np.asarray(bi, np.float32)
    Wh = np.asarray(Wh, np.float32)
    bhv = np.asarray(bh, np.float32)
    Wo = np.asarray(Wo, np.float32).copy()
    bo = np.asarray(bo, np.float32).copy()
    # driftV net (index 1) is only ever used multiplied by h -> fold h into it
    Wo[1] *= h
    bo0, bo1h, bo2, bo3 = float(bo[0, 0]), float(bo[1, 0]) * h, float(bo[2, 0]), float(bo[3, 0])

    cS, cV = S0v, V0v    # centering constants for fp16 inputs
    # first-layer bias with t-term and centering folded in: [4, T, 128]
    t_vals = tg[:n_steps].astype(np.float32)
    b1 = (bi[:, None, :] + t_vals[None, :, None] * Wi[:, 0][:, None, :]
          + cS * Wi[:, 1][:, None, :] + cV * Wi[:, 2][:, None, :])

    # layer-1 stationary per (t, n): rows (Wi_S, Wi_V, b1(t)): [3, T*4*128]
    wiT3 = np.empty((3, n_steps, 4, 128), np.float32)
    wiT3[0] = np.broadcast_to(Wi[:, 1, :][None, :, :], (n_steps, 4, 128))
    wiT3[1] = np.broadcast_to(Wi[:, 2, :][None, :, :], (n_steps, 4, 128))
    wiT3[2] = b1.transpose(1, 0, 2)
    wiT3_dev = np.ascontiguousarray(wiT3.reshape(3, n_steps * 512), np.float16)

    whT_dev = np.ascontiguousarray(
        Wh.transpose(2, 0, 1, 3).reshape(128, 12 * 128), np.float16)
    woT_dev = np.ascontiguousarray(Wo[:, :, 0].T, np.float16)
    bh_dev = np.ascontiguousarray(bhv.transpose(2, 0, 1).reshape(128, 12), np.float32)

    strk_dev = np.ascontiguousarray(
        np.tile(np.concatenate([-STRIKES_CALL, STRIKES_PUT,
                                -STRIKES_PUT, STRIKES_CALL])[None, :], (128, 1)),
        np.float32)

    idx = np.asarray(indices).astype(np.int64).reshape(-1)
    idx_steps = [int((v - 1) % n_steps) for v in idx]
    disc = np.exp(-r * 2.0 * idx.astype(np.float64) / n_steps).astype(np.float64)

    z = np.asarray(z, np.float32)
    z1 = np.asarray(z1, np.float32)
    in_maps = []
    for kk in range(N_CORES):
        sl = slice(kk * MCL, (kk + 1) * MCL)
        # T-layout per step: dev[t*128+d, m] = sqh * z[sample m*128+d, t]
        zc = (z[sl, :n_steps] * sqh).T.reshape(n_steps * 128, 128)
        z1c = (z1[sl, :n_steps] * sqh).T.reshape(n_steps * 128, 128)
        in_maps.append({
            "z": np.ascontiguousarray(zc, np.float32),
            "z1": np.ascontiguousarray(z1c, np.float32),
            "wiT3": wiT3_dev, "whT": whT_dev, "woT": woT_dev, "bh": bh_dev,
            "strk": strk_dev,
        })
    build_args = dict(idx_steps=idx_steps, c0=c0, bo0=bo0, bo1h=bo1h,
                      bo2=bo2, bo3=bo3, cS=cS, cV=cV, n_steps=n_steps)
    return build_args, in_maps, disc


def _combine(results, disc):
    """Sum per-core per-engine [128, 960] partials into the [96, 10] output.

    Device provides relu(S-Kc) (cols 0-9), relu(Kp-S) (cols 10-19) and
    sum(S) (col 20) per maturity; the other two payoff families follow from
    relu(x) - relu(-x) = x summed over samples.
    """
    total = np.zeros((128, 960), np.float64)
    for res in results:
        for k in ("accA", "accD", "accP"):
            total += np.asarray(res[k], np.float64)
    cols = total.sum(axis=0).reshape(24, 40)
    calls_c = cols[:, 0:10]
    puts_p = cols[:, 10:20]
    sumS = cols[:, 20]
    Kc = STRIKES_CALL.astype(np.float64)
    Kp = STRIKES_PUT.astype(np.float64)
    calls_p = puts_p + sumS[:, None] - MC * Kp[None, :]
    puts_c = calls_c - sumS[:, None] + MC * Kc[None, :]
    out = np.concatenate([calls_c, puts_p, calls_p, puts_c], axis=0) / MC
    out = out * np.concatenate([disc] * 4)[:, None]
    return out.astype(np.float32)


def kernel(**inputs) -> np.ndarray:
    from concourse.bass_utils import run_bass_kernel_spmd
    _install_sync_split()
    build_args, in_maps, disc = _prep_inputs(**inputs)
    nc = build_nc(**build_args)
    res = run_bass_kernel_spmd(nc, in_maps, list(range(N_CORES)))
    return _combine(res.results, disc)
